# revision 26
# baseline (speedup 1.0000x reference)
"""BitLinear-1.58 Trainium2 kernel (8-core SPMD), v7.

out = (clip(round(x * s), -128, 127) @ w.T) / s / weight_scale + bias,
s = 127 / clip(rowmax|x|, 1e-5),  w in {0,1} (int32), x [4096, 8192] f32.

Sharding: token dim split 4 ways x out-feature dim split 2 ways -> 8 cores.
Each core: x-block [1024, 8192], weight-block [4096, 8192], out-block [1024, 4096].

Lessons encoded here (from HW traces of v1-v8):
  * GpSimd compute (tensor_scalar ~29us, DSP launch ~14us) blocks its DMA
    queue — GpSimd issues DMAs only; broadcasts ride stride-0 DMA reads.
  * In-place size-shrinking writes (f32 tile -> bf16 front half) are safe on
    ACT (serial) but NOT on DVE: DVE splits the free dim across parallel
    streams and the upper stream's writes clobber the lower stream's unread
    input (observed: cols 512..1023 of [128,2048] corrupted).
  * Cross-engine hops cost ~2-5us and DMA completions become visible ~5us
    (loads) to ~10us (XBAR) late; pipeline throughput = ring_latency /
    ring_bufs for every buffer-recycle ring. phx (x load->...->round) and
    phxq (round->XBAR-read) sized accordingly; adding MORE stage pools at
    shallower depth is a net loss (v8 regression).
  * The drain staging pool (outp) must cycle 8 tiles so a drain never waits
    on a store completion from the same nt burst (v5 lost ~20us per nt
    exactly there); staging is bf16 (<= ~4e-3 relative error, gate is
    2e-2) to afford the depth. ob is bf16; the host widens to f32.
  * Matmul loops run t-outer so each psum's final matmul lands 16 matmuls
    before the tile's drain is needed, hiding drains behind tail matmuls.
  * The scheduler list-schedules each engine queue (emission order is only
    a hint); pacing is set by ring depths and the semaphore graph.
  * Tile pools are live-range allocated: Phase X borrows the weight
    pipeline's SBUF (phx+phxq close before wnat/slab open).

Schedule:
  Phase X (~230us): x quarters stream on Sync+GpSimd queues; DVE rowmax ->
    s = 127*recip(m) (approximate s only flips round() at half-ulp
    boundaries; the output scale d = m/127/ws is computed exactly from m);
    ACT does fused x*s+MAGIC in place; -MAGIC->bf16 rounds split ACT/DVE
    into phxq; Sync XBAR-transposes into the resident xq
    [128k,64ko,1024t] bf16 cache (16 MB).
  GEMM (~1.1ms, TensorE ~93% busy): weight streams as [128n,2048k] int32
    chunks on Sync, ACT casts to bf16 in place, Sync XBAR into
    [128k,16ko,512n] slabs (bufs=3, prepped 2 groups ahead). Per (nt,t):
    psum [128t,512n] accumulates 64 ko; DVE drains (scale by per-token d,
    add bias) into bf16 staging, GpSimd stores.

Exactness: x_q ints in [-127,127] and w {0,1} are exact in bf16; every
partial sum < 2^24 so fp32 PSUM accumulation is exact. round() = +M,-M
magic (RNE, matches jnp.round). clip never binds since |x*s| <= 127.
"""
import sys

sys.path.insert(0, "/opt/trn_rl_repo")

from contextlib import ExitStack

import numpy as np

import concourse.bass as bass
import concourse.tile as tile
from concourse import bacc, mybir
from concourse.bass import ts
from concourse.bass_utils import run_bass_kernel_spmd

TOKENS, IN_F, OUT_F = 4096, 8192, 8192
A_SPLIT, B_SPLIT = 4, 2      # token blocks x outfeature blocks = 8 cores
T_LOC = TOKENS // A_SPLIT    # 1024
N_LOC = OUT_F // B_SPLIT     # 4096
P = 128
KO = IN_F // P               # 64 k-tiles of 128
TT = T_LOC // P              # 8 token tiles
NT = N_LOC // 512            # 8 n-tiles of 512
KQ = 4                       # k quarters (16 ko each) per slab group
KO_Q = KO // KQ              # 16
NB = 4                       # 128-row n blocks per 512 n-tile
WQW = 2048                   # w chunk width in k
XHW = 2048                   # x chunk width in k
XHN = IN_F // XHW            # 4 quarters
MAGIC = float(np.float32(1.5 * 2 ** 23))

_CACHE = {}


def _build():
    if "nc" in _CACHE:
        return _CACHE["nc"]

    nc = bacc.Bacc("TRN2", target_bir_lowering=False, debug=False, num_devices=8)
    f32, bf16, i32 = mybir.dt.float32, mybir.dt.bfloat16, mybir.dt.int32
    A = mybir.AluOpType

    xb = nc.dram_tensor("xb", [T_LOC, IN_F], f32, kind="ExternalInput").ap()
    wb = nc.dram_tensor("wb", [N_LOC, IN_F], i32, kind="ExternalInput").ap()
    bb = nc.dram_tensor("bb", [N_LOC], f32, kind="ExternalInput").ap()
    ws = nc.dram_tensor("ws", [1], f32, kind="ExternalInput").ap()
    ob = nc.dram_tensor("ob", [T_LOC, N_LOC], bf16, kind="ExternalOutput").ap()

    with tile.TileContext(nc) as tc:
        with ExitStack() as ctx:
            small = ctx.enter_context(tc.tile_pool(name="small", bufs=1))
            sp2 = ctx.enter_context(tc.tile_pool(name="sp2", bufs=2))
            xqp = ctx.enter_context(tc.tile_pool(name="xq", bufs=1))
            pp = ctx.enter_context(tc.tile_pool(name="psum", bufs=8, space="PSUM"))

            xq = xqp.tile([P, KO, T_LOC], bf16)   # resident 16 MB xq cache

            # ---- prologue (DMA-only; no gpsimd DSP ops) ----
            rws_b = small.tile([P, 1], f32)
            nc.gpsimd.dma_start(rws_b[:], ws[None, :].broadcast_to((P, 1)))
            nc.vector.reciprocal(rws_b[:], rws_b[:])

            d_all = small.tile([P, TT], f32)      # per-token out scale 1/s/ws
            m_all = small.tile([P, TT], f32)

            # ---- Phase X: x -> s -> quantize -> XBAR into xq cache ----
            # ring depths: a phx buffer cycles load->reduce->scale->round
            # (compute-fast); a phxq buffer cycles round->XBAR-read whose
            # completion is ~10us late to become visible. Both rings need
            # >= 1.25 tiles of slack or the recycle latency sets the pace.
            with tc.tile_pool(name="phx", bufs=7) as phx, \
                 tc.tile_pool(name="phxq", bufs=5) as phq:
                for tt in range(TT):
                    m4 = sp2.tile([P, XHN], f32, tag="m4")
                    quarters = []
                    for q in range(XHN):
                        xh = phx.tile([P, XHW], f32, tag="xh")
                        # 2+2 queue split: one DMA queue alone serializes the
                        # 4x2.9us quarter loads and paces the whole tile ring
                        eng = nc.sync if q % 2 == 0 else nc.gpsimd
                        eng.dma_start(xh[:], xb[ts(tt, P), ts(q, XHW)])
                        nc.vector.tensor_reduce(
                            m4[:, q : q + 1], xh[:], mybir.AxisListType.X,
                            A.max, apply_absolute_value=True)
                        quarters.append(xh)
                    m_col = m_all[:, tt : tt + 1]
                    nc.vector.tensor_reduce(m_col, m4[:],
                                            mybir.AxisListType.X, A.max)
                    nc.vector.tensor_scalar_max(m_col, m_col, 1e-5)
                    s_t = sp2.tile([P, 1], f32, tag="s_t")
                    nc.vector.reciprocal(s_t[:], m_col)
                    nc.vector.tensor_scalar_mul(s_t[:], s_t[:], 127.0)
                    # d = m / 127 / ws  (exact chain, independent of s)
                    nc.vector.tensor_scalar(d_all[:, tt : tt + 1], m_col,
                                            rws_b[:, 0:1], 1.0 / 127.0,
                                            A.mult, A.mult)
                    for xh in quarters:
                        # ACT: xh = x*s + MAGIC in place (f32, exact int part)
                        nc.scalar.activation(xh[:], xh[:],
                                             mybir.ActivationFunctionType.Copy,
                                             bias=MAGIC, scale=s_t[:, 0:1])
                    for q, xh in enumerate(quarters):
                        # -MAGIC -> bf16 staging; split across DVE and ACT
                        xqh = phq.tile([P, XHW], bf16, tag="xqh")
                        if q % 2 == 0:
                            nc.vector.tensor_scalar_sub(xqh[:], xh[:], MAGIC)
                        else:
                            nc.scalar.activation(
                                xqh[:], xh[:],
                                mybir.ActivationFunctionType.Copy, bias=-MAGIC)
                        nc.sync.dma_start_transpose(
                            xq[:, ts(q, KO // XHN), ts(tt, P)], xqh[:])

            # ---- GEMM: stream w, matmul, drain ----
            wnp = ctx.enter_context(tc.tile_pool(name="wnat", bufs=3))
            wcp = ctx.enter_context(tc.tile_pool(name="wcvt", bufs=4))
            slp = ctx.enter_context(tc.tile_pool(name="slab", bufs=3))
            op = ctx.enter_context(tc.tile_pool(name="outp", bufs=8))
            # bbc opens here (not in the prologue) so its live range does not
            # overlap Phase X — that SBUF is phx's 7th buffer.
            bbp = ctx.enter_context(tc.tile_pool(name="bbc", bufs=1))

            b_bcs = {}

            def emit_bias(nt):
                b_bc = bbp.tile([P, 512], f32, tag="bbc", name=f"bbc_{nt}")
                nc.gpsimd.dma_start(
                    b_bc[:], bb[None, ts(nt, 512)].broadcast_to((P, 512)))
                b_bcs[nt] = b_bc

            emit_bias(0)

            slabs = {}

            def emit_slab(nt, kq):
                # half-width chunks with separate cast staging: the int32
                # buffer recycles at cast-read (fast, compute-visible) and
                # only the small wcvt ring carries the ~10us XBAR-completion
                # lag; both rings then beat the 27us/group demand pitch.
                slab = slp.tile([P, KO_Q, 512], bf16, tag="slab",
                                name=f"slab_{nt}_{kq}")
                slabs[(nt, kq)] = slab
                for nb in range(NB):
                    for h in range(2):
                        w_i = wnp.tile([P, WQW // 2], i32, tag="wi",
                                       name=f"wi_{nt}_{kq}_{nb}_{h}")
                        nc.sync.dma_start(
                            w_i[:], wb[ts(nt * NB + nb, P),
                                       kq * WQW + h * (WQW // 2)
                                       : kq * WQW + (h + 1) * (WQW // 2)])
                        w_c = wcp.tile([P, WQW // 2], bf16, tag="wc",
                                       name=f"wc_{nt}_{kq}_{nb}_{h}")
                        nc.scalar.copy(w_c[:], w_i[:])
                        nc.sync.dma_start_transpose(
                            slab[:, h * (KO_Q // 2) : (h + 1) * (KO_Q // 2),
                                 ts(nb, P)], w_c[:])

            emit_slab(0, 0)
            emit_slab(0, 1)

            psums = {}
            for i in range(NT * KQ):
                nt, kq = divmod(i, KQ)
                if i + 2 < NT * KQ:
                    pnt, pkq = divmod(i + 2, KQ)
                    emit_slab(pnt, pkq)
                if kq == 1 and nt + 1 < NT:
                    emit_bias(nt + 1)
                if kq == 0:
                    psums[nt] = [pp.tile([P, 512], f32, tag="acc",
                                         name=f"ps_{nt}_{t}")
                                 for t in range(TT)]
                slab = slabs.pop((nt, kq))
                last = kq == KQ - 1
                if last:
                    b_bc = b_bcs.pop(nt)
                for t in range(TT):
                    for kol in range(KO_Q):
                        ko = kq * KO_Q + kol
                        nc.tensor.matmul(
                            psums[nt][t][:], xq[:, ko, ts(t, P)],
                            slab[:, kol, :],
                            start=(ko == 0), stop=(ko == KO - 1))
                    if last:
                        # drain right behind this tile's final matmul on
                        # DVE+GpSimd; bf16 staging (<= 2 final roundings)
                        o_sb = op.tile([P, 512], bf16, tag="osb",
                                       name=f"osb_{nt}_{t}")
                        nc.vector.tensor_scalar(o_sb[:], psums[nt][t][:],
                                                d_all[:, t : t + 1], None,
                                                A.mult)
                        nc.vector.tensor_tensor(o_sb[:], o_sb[:], b_bc[:],
                                                A.add)
                        nc.gpsimd.dma_start(ob[ts(t, P), ts(nt, 512)], o_sb[:])
                if last:
                    del psums[nt]

    nc.compile()
    _CACHE["nc"] = nc
    return nc


def kernel(x, weight, weight_scale, bias):
    x = np.ascontiguousarray(np.asarray(x, dtype=np.float32))
    weight = np.ascontiguousarray(np.asarray(weight, dtype=np.int32))
    weight_scale = np.asarray(weight_scale, dtype=np.float32).reshape(1)
    bias = np.ascontiguousarray(np.asarray(bias, dtype=np.float32))

    nc = _build()
    in_maps = []
    for c in range(8):
        i, j = c // B_SPLIT, c % B_SPLIT
        in_maps.append({
            "xb": x[i * T_LOC:(i + 1) * T_LOC],
            "wb": weight[j * N_LOC:(j + 1) * N_LOC],
            "bb": bias[j * N_LOC:(j + 1) * N_LOC],
            "ws": weight_scale,
        })
    res = run_bass_kernel_spmd(nc, in_maps, list(range(8))).results

    out = np.empty((TOKENS, OUT_F), dtype=np.float32)
    for c in range(8):
        i, j = c // B_SPLIT, c % B_SPLIT
        out[i * T_LOC:(i + 1) * T_LOC,
            j * N_LOC:(j + 1) * N_LOC] = np.asarray(res[c]["ob"],
                                                    dtype=np.float32)
    return out
